# revision 7
# baseline (speedup 1.0000x reference)
"""GCN (2-layer + linear classifier) on 8 Trainium2 NeuronCores.

Math: with A = adjacency+self-loops and dis = deg^-1/2 (deg over incoming
edges incl. self-loops), PyG gcn_norm gives norm_e = dis[src]*dis[dst], which
is separable. So each conv layer is
    out = dis ⊙ (A_binary @ ((dis ⊙ h) @ W)) + b
i.e. a plain binary-adjacency segment-sum around a dense matmul — no per-edge
scaling.

Distribution (8 cores): nodes are split into 8 contiguous chunks; edges are
partitioned by destination-node owner (segment-sum is local); layer-1 h-tilde
is computed replicated from the full x; the single cross-core exchange is one
fp16 AllGather of the layer-2 message table.

Per core, the aggregation runs per 128-destination-node window: source rows
are fetched from a DRAM fp16 table with dma_gather (128 rows/tile), reduced
onto a PSUM accumulator with TensorEngine matmuls against one-hot selection
matrices built by a DVE is_equal, then the epilogue (dis-scale, bias, relu,
next-layer matmul) runs on DVE/ACT/PE. Node indices are split at row 32768
into lo/hi tables because dma_gather indices are int16.
"""
import os
import numpy as np
import ml_dtypes

import concourse.bacc as bacc
import concourse.bass as bass
import concourse.mybir as mybir
import concourse.tile as tile
from concourse import library_config
from concourse.bass_utils import run_bass_kernel_spmd

N_CORES = 8
D = 128           # feature dim (= hidden dim = partition count)
LO_DEFAULT = 32768

fp16 = mybir.dt.float16
f32 = mybir.dt.float32
i16 = mybir.dt.int16


# ---------------------------------------------------------------- host prep

def _wrap16(v):
    """dma_gather index layout: idx i -> partition i%16, col i//16,
    replicated across all eight 16-partition groups."""
    a = v.reshape(-1, 16).T.astype(np.int16)
    return np.tile(a, (8, 1))


def prep(x, edge_index, n_cores=N_CORES, lo_rows=LO_DEFAULT):
    N = x.shape[0]
    locN = N // n_cores
    assert locN * n_cores == N
    WPC = -(-locN // 128)              # real (dst) windows per core
    NLOC = -(-locN // 512) * 512       # padded nodes per core (512-slab aligned)
    NPAD = n_cores * NLOC
    assert lo_rows % 128 == 0 and lo_rows < 32768 + 1

    src = np.asarray(edge_index[0]).astype(np.int64)
    dst = np.asarray(edge_index[1]).astype(np.int64)
    loop = np.arange(N, dtype=np.int64)
    src_all = np.concatenate([src, loop])
    dst_all = np.concatenate([dst, loop])

    deg = np.bincount(dst_all, minlength=N).astype(np.float32)

    d_core = dst_all // locN
    d_off = dst_all - d_core * locN
    w_global = d_core * WPC + d_off // 128
    wrow = (d_off % 128).astype(np.float16)
    spid = (src_all // locN) * NLOC + (src_all % locN)
    srow = (spid // 512) * 512 + (spid % 128) * 4 + (spid % 512) // 128
    hi_flag = (srow >= lo_rows).astype(np.int64)

    key = w_global * 2 + hi_flag
    order = np.argsort(key, kind="stable")
    counts = np.bincount(key, minlength=n_cores * WPC * 2).reshape(n_cores, WPC, 2)
    offs = np.concatenate([[0], np.cumsum(counts.reshape(-1))]).astype(np.int64)

    # shared (max-across-cores) tile structure
    T_lo = [int(-(-counts[:, w, 0].max() // 128)) for w in range(WPC)]
    T_hi = [int(-(-counts[:, w, 1].max() // 128)) for w in range(WPC)]

    spid_sorted = srow[order]
    wrow_sorted = wrow[order]

    per_core = []
    for c in range(n_cores):
        ilo_parts, ihi_parts, wr_parts = [], [], []
        for w in range(WPC):
            base = (c * WPC + w) * 2
            for h, T in ((0, T_lo[w]), (1, T_hi[w])):
                n = T * 128
                if n == 0:
                    continue
                a, b = offs[base + h], offs[base + h + 1]
                sp = spid_sorted[a:b]
                wr = wrow_sorted[a:b]
                pad = n - (b - a)
                idx = np.concatenate([sp - (lo_rows if h else 0),
                                      np.zeros(pad, np.int64)]).astype(np.int16)
                wrc = np.concatenate([wr, np.full(pad, -1.0, np.float16)])
                (ihi_parts if h else ilo_parts).append(idx)
                wr_parts.append(wrc.reshape(T, 128).T)
        idx_lo = _wrap16(np.concatenate(ilo_parts)) if ilo_parts else np.zeros((128, 8), np.int16)
        idx_hi = _wrap16(np.concatenate(ihi_parts)) if ihi_parts else np.zeros((128, 8), np.int16)
        wrow_c = np.concatenate(wr_parts, axis=1).astype(np.float16)

        # per-core deg row over its padded local nodes (pads get deg 1)
        dr = np.ones((1, NLOC), np.float32)
        dr[0, :locN] = deg[c * locN:(c + 1) * locN]
        per_core.append(dict(idx_lo=idx_lo, idx_hi=idx_hi, wrow=wrow_c, deg_row=dr))

    # x-tilde transposed, fp16: (dis * x) laid out [D, NPAD] in padded-id order
    dis = 1.0 / np.sqrt(deg)
    xt = np.zeros((D, NPAD), np.float16)
    xs = (np.asarray(x, np.float32) * dis[:, None]).astype(np.float16)
    for c in range(n_cores):
        xt[:, c * NLOC: c * NLOC + locN] = xs[c * locN:(c + 1) * locN].T

    struct = dict(N=N, locN=locN, WPC=WPC, NLOC=NLOC, NPAD=NPAD,
                  lo_rows=lo_rows, T_lo=tuple(T_lo), T_hi=tuple(T_hi),
                  n_cores=n_cores)
    return struct, per_core, xt


# ------------------------------------------------------------- bass program

def build(struct):
    WPC, NLOC, NPAD = struct["WPC"], struct["NLOC"], struct["NPAD"]
    LO = struct["lo_rows"]
    T_lo, T_hi = struct["T_lo"], struct["T_hi"]
    n_cores = struct["n_cores"]
    CL = max(8, 8 * sum(T_lo))
    CH = max(8, 8 * sum(T_hi))
    TT = sum(T_lo) + sum(T_hi)
    maxT = max(T_lo[w] + T_hi[w] for w in range(WPC))
    nblk = NPAD // 128

    nc = bacc.Bacc("TRN2", target_bir_lowering=False, debug=False,
                   num_devices=n_cores, num_swdge_queues=4)
    xt_d = nc.dram_tensor("xt", [D, NPAD], fp16, kind="ExternalInput")
    W1_d = nc.dram_tensor("W1", [D, D], f32, kind="ExternalInput")
    W2_d = nc.dram_tensor("W2", [D, D], f32, kind="ExternalInput")
    Wc_d = nc.dram_tensor("Wc", [D, 2], f32, kind="ExternalInput")
    b1_d = nc.dram_tensor("b1c", [D, 1], f32, kind="ExternalInput")
    b2_d = nc.dram_tensor("b2c", [D, 1], f32, kind="ExternalInput")
    bc_d = nc.dram_tensor("bcrep", [D, 2], f32, kind="ExternalInput")
    iota_d = nc.dram_tensor("iota", [D, D], fp16, kind="ExternalInput")
    deg_d = nc.dram_tensor("deg_row", [1, NLOC], f32, kind="ExternalInput")
    ilo_d = nc.dram_tensor("idx_lo", [128, CL], i16, kind="ExternalInput")
    ihi_d = nc.dram_tensor("idx_hi", [128, CH], i16, kind="ExternalInput")
    wrow_d = nc.dram_tensor("wrow", [128, TT], fp16, kind="ExternalInput")
    out_d = nc.dram_tensor("out", [NLOC, 2], f32, kind="ExternalOutput")

    htab1 = nc.dram_tensor("htab1", [NPAD, D], fp16)
    ag_in = nc.dram_tensor("ag_in", [NLOC, D], fp16)
    htab2 = nc.dram_tensor("htab2", [NPAD, D], fp16, addr_space="Shared")

    with tile.TileContext(nc) as tc:
        nc.gpsimd.load_library(library_config.mlp)
        with (
            tc.tile_pool(name="const", bufs=1) as cp,
            tc.tile_pool(name="work", bufs=3) as wp,
            tc.tile_pool(name="msgp", bufs=2) as mp,
            tc.tile_pool(name="Sp", bufs=4) as sp_,
            tc.tile_pool(name="psum", bufs=2, space="PSUM") as pp,
        ):
            # ---- constants
            W1s = cp.tile([D, D], fp16)
            W2s = cp.tile([D, D], fp16)
            Wcs = cp.tile([D, 2], fp16)
            nc.gpsimd.dma_start(out=W1s[:], in_=W1_d[:])   # SWDGE casts f32->fp16
            nc.gpsimd.dma_start(out=W2s[:], in_=W2_d[:])
            nc.gpsimd.dma_start(out=Wcs[:], in_=Wc_d[:])
            b1c = cp.tile([D, 1], f32)
            b2c = cp.tile([D, 1], f32)
            bcr = cp.tile([D, 2], f32)
            iota = cp.tile([D, D], fp16)
            nc.sync.dma_start(out=b1c[:], in_=b1_d[:])
            nc.sync.dma_start(out=b2c[:], in_=b2_d[:])
            nc.sync.dma_start(out=bcr[:], in_=bc_d[:])
            nc.sync.dma_start(out=iota[:], in_=iota_d[:])
            ilo = cp.tile([128, CL], i16)
            ihi = cp.tile([128, CH], i16)
            wro = cp.tile([128, TT], fp16)
            nc.sync.dma_start(out=ilo[:], in_=ilo_d[:])
            nc.sync.dma_start(out=ihi[:], in_=ihi_d[:])
            nc.sync.dma_start(out=wro[:], in_=wrow_d[:])

            # dis row + replicated dis  (dis = 1/sqrt(deg))
            degr = cp.tile([1, NLOC], f32)
            nc.sync.dma_start(out=degr[:], in_=deg_d[:])
            sq = cp.tile([1, NLOC], f32)
            nc.scalar.activation(sq[:], degr[:], mybir.ActivationFunctionType.Sqrt)
            disr = cp.tile([1, NLOC], f32)
            nc.vector.reciprocal(out=disr[:], in_=sq[:])
            ones1 = cp.tile([1, 128], f32)
            nc.vector.memset(ones1[:], 1.0)
            disrep = cp.tile([128, NLOC], f32)
            c0 = 0
            while c0 < NLOC:
                cw = min(512, NLOC - c0)
                ps = pp.tile([128, 512], f32, space="PSUM", tag="mm")
                nc.tensor.matmul(out=ps[:, :cw], lhsT=ones1[:],
                                 rhs=disr[0:1, c0:c0 + cw], start=True, stop=True)
                nc.vector.tensor_copy(out=disrep[:, c0:c0 + cw], in_=ps[:, :cw])
                c0 += cw

            # ---- P1: full h1-tilde table, replicated on every core
            SLAB = 4  # 128-col blocks per DMA slab
            for s0 in range(0, nblk, SLAB):
                sb = min(SLAB, nblk - s0)
                xts = wp.tile([128, SLAB, 128], fp16, tag="xts")
                nc.sync.dma_start(out=xts[:, :sb, :],
                                  in_=xt_d[:, s0 * 128:(s0 + sb) * 128]
                                  .rearrange("k (a d) -> k a d", a=sb))
                hs = wp.tile([128, SLAB, 128], fp16, tag="hout")
                ps = pp.tile([128, 512], f32, space="PSUM", tag="mm")
                for j in range(sb):
                    nc.tensor.matmul(out=ps[:, j * 128:(j + 1) * 128],
                                     lhsT=xts[:, j, :], rhs=W1s[:],
                                     start=True, stop=True)
                nc.vector.tensor_copy(
                    out=hs[:, :sb, :],
                    in_=ps[:, :sb * 128].rearrange("p (a d) -> p a d", a=sb))
                nc.sync.dma_start(
                    out=htab1[s0 * 128:(s0 + sb) * 128, :]
                    .rearrange("(p j) d -> p j d", p=128),
                    in_=hs[:, :sb, :])

            # ---- one aggregation layer over all windows
            def layer(tab, emit_window):
                clo = chi = ct = 0
                qn = [0]
                for w in range(WPC):
                    tl, th = T_lo[w], T_hi[w]
                    Tw = tl + th
                    msg = mp.tile([128, maxT, 128], fp16, tag="msg")
                    # single_packet coalesces a gather's whole descriptor
                    # stream into one SDMA packet; packets cap at 64
                    # descriptors (8 per tile per engine), so chunk to <=7
                    # tiles per dma_gather.
                    GMAX = 7
                    for t0 in range(0, tl, GMAX):
                        tc_ = min(GMAX, tl - t0)
                        nc.gpsimd.dma_gather(
                            msg[:, t0:t0 + tc_, :], tab[0:LO, :],
                            ilo[:, clo + t0 * 8:clo + (t0 + tc_) * 8],
                            tc_ * 128, tc_ * 128, D, queue_num=qn[0] % 4)
                        qn[0] += 1
                    for t0 in range(0, th, GMAX):
                        tc_ = min(GMAX, th - t0)
                        nc.gpsimd.dma_gather(
                            msg[:, tl + t0:tl + t0 + tc_, :], tab[LO:, :],
                            ihi[:, chi + t0 * 8:chi + (t0 + tc_) * 8],
                            tc_ * 128, tc_ * 128, D, queue_num=qn[0] % 4)
                        qn[0] += 1
                    pa = pp.tile([128, 128], f32, space="PSUM", tag="agg")
                    SG = 8
                    for g0 in range(0, Tw, SG):
                        gk = min(SG, Tw - g0)
                        S = sp_.tile([128, SG * 128], fp16, tag="S")
                        iap = iota[:]
                        iota_b = bass.AP(iap.tensor, iap.offset,
                                         [iap.ap[0], [0, gk], iap.ap[1]])
                        nc.vector.tensor_tensor(
                            out=S[:, :gk * 128].rearrange("p (t d) -> p t d", t=gk),
                            in0=wro[:, ct + g0:ct + g0 + gk].to_broadcast([128, gk, 128]),
                            in1=iota_b, op=mybir.AluOpType.is_equal)
                        for t in range(g0, g0 + gk):
                            ts_ = t - g0
                            nc.tensor.matmul(
                                out=pa[:], lhsT=msg[:, t, :],
                                rhs=S[:, ts_ * 128:(ts_ + 1) * 128],
                                start=(t == 0), stop=(t == Tw - 1))
                    emit_window(w, pa)
                    clo += tl * 8
                    chi += th * 8
                    ct += Tw

            # layer 1 window epilogue: h2 = relu(dis*agg + b1); y = dis*h2;
            # htilde2 = y^T @ W2  -> ag_in rows
            def epi1(w, pa):
                dw = disrep[:, w * 128:(w + 1) * 128]
                z = wp.tile([128, 128], f32, tag="z")
                nc.vector.tensor_mul(out=z[:], in0=pa[:], in1=dw)
                h2 = wp.tile([128, 128], f32, tag="h2")
                nc.scalar.activation(h2[:], z[:], mybir.ActivationFunctionType.Relu,
                                     bias=b1c[:, 0:1], scale=1.0)
                y = wp.tile([128, 128], fp16, tag="y")
                nc.vector.tensor_mul(out=y[:], in0=h2[:], in1=dw)
                p2 = pp.tile([128, 128], f32, space="PSUM", tag="mm")
                nc.tensor.matmul(out=p2[:], lhsT=y[:], rhs=W2s[:], start=True, stop=True)
                hb = wp.tile([128, 128], fp16, tag="hb")
                nc.vector.tensor_copy(out=hb[:], in_=p2[:])
                agv = ag_in[:].rearrange("(s p j) d -> s p j d", p=128, j=4)
                nc.sync.dma_start(out=agv[w // 4, :, w % 4, :], in_=hb[:])

            layer(htab1, epi1)

            # zero the pad window-slots of ag_in (NLOC is 512-aligned but only
            # WPC windows are real) so the AllGather ships finite data
            if NLOC // 128 > WPC:
                zt = cp.tile([128, 128], fp16)
                nc.vector.memset(zt[:], 0.0)
                agv0 = ag_in[:].rearrange("(s p j) d -> s p j d", p=128, j=4)
                for w in range(WPC, NLOC // 128):
                    nc.sync.dma_start(out=agv0[w // 4, :, w % 4, :], in_=zt[:])

            nc.gpsimd.collective_compute(
                "AllGather", mybir.AluOpType.bypass,
                replica_groups=[list(range(n_cores))],
                ins=[ag_in.ap().opt()], outs=[htab2.ap().opt()])

            # layer 2 window epilogue: out3 = dis*agg + b2 ; out = out3^T@Wc + bc
            outacc = cp.tile([128, WPC, 2], f32)

            def epi2(w, pa):
                dw = disrep[:, w * 128:(w + 1) * 128]
                z = wp.tile([128, 128], f32, tag="z2")
                nc.vector.tensor_mul(out=z[:], in0=pa[:], in1=dw)
                o3 = wp.tile([128, 128], fp16, tag="o3")
                nc.scalar.activation(o3[:], z[:], mybir.ActivationFunctionType.Identity,
                                     bias=b2c[:, 0:1], scale=1.0)
                p3 = pp.tile([128, 2], f32, space="PSUM", tag="cls")
                nc.tensor.matmul(out=p3[:], lhsT=o3[:], rhs=Wcs[:], start=True, stop=True)
                nc.vector.tensor_add(out=outacc[:, w, :], in0=p3[:], in1=bcr[:])

            layer(htab2, epi2)
            nc.sync.dma_start(
                out=out_d[:WPC * 128, :].rearrange("(w p) c -> p w c", p=128),
                in_=outacc[:])

    nc.compile()
    return nc


# ------------------------------------------------------------------ driver

_CACHE = {}


def _get_program(struct):
    key = tuple(sorted((k, v) for k, v in struct.items()))
    if key not in _CACHE:
        _CACHE[key] = build(struct)
    return _CACHE[key]


def kernel(x, edge_index, W1, b1, W2, b2, Wc, bc):
    x = np.asarray(x)
    N = x.shape[0]
    struct, per_core, xt = prep(x, edge_index)
    nc = _get_program(struct)
    locN, NLOC = struct["locN"], struct["NLOC"]

    common = dict(
        xt=xt,
        W1=np.asarray(W1, np.float32),
        W2=np.asarray(W2, np.float32),
        Wc=np.asarray(Wc, np.float32),
        b1c=np.asarray(b1, np.float32).reshape(D, 1),
        b2c=np.asarray(b2, np.float32).reshape(D, 1),
        bcrep=np.tile(np.asarray(bc, np.float32).reshape(1, 2), (D, 1)),
        iota=np.tile(np.arange(D, dtype=np.float16), (D, 1)),
    )
    in_maps = []
    for c in range(N_CORES):
        m = dict(common)
        m["deg_row"] = per_core[c]["deg_row"]
        m["idx_lo"] = per_core[c]["idx_lo"]
        m["idx_hi"] = per_core[c]["idx_hi"]
        m["wrow"] = per_core[c]["wrow"]
        in_maps.append(m)

    trace = bool(int(os.environ.get("KERNEL_TRACE", "0")))
    res = run_bass_kernel_spmd(nc, in_maps, core_ids=list(range(N_CORES)),
                               trace=trace)
    if trace and res.exec_time_ns is not None:
        print(f"HW exec time: {res.exec_time_ns} ns", flush=True)

    out = np.empty((N, 2), np.float32)
    for c in range(N_CORES):
        out[c * locN:(c + 1) * locN] = res.results[c]["out"][:locN]
    return out


# revision 9
# speedup vs baseline: 1.0681x; 1.0681x over previous
"""GCN (2-layer + linear classifier) on 8 Trainium2 NeuronCores.

Math: with A = adjacency+self-loops and dis = deg^-1/2 (deg over incoming
edges incl. self-loops), PyG gcn_norm gives norm_e = dis[src]*dis[dst], which
is separable. So each conv layer is
    out = dis ⊙ (A_binary @ ((dis ⊙ h) @ W)) + b
i.e. a plain binary-adjacency segment-sum around a dense matmul — no per-edge
scaling.

Distribution (8 cores): nodes are split into 8 contiguous chunks; edges are
partitioned by destination-node owner (segment-sum is local); layer-1 h-tilde
is computed replicated from the full x; the single cross-core exchange is one
fp16 AllGather of the layer-2 message table.

Per core, the aggregation runs per 128-destination-node window: source rows
are fetched from a DRAM fp16 table with dma_gather (128 rows/tile), reduced
onto a PSUM accumulator with TensorEngine matmuls against one-hot selection
matrices built by a DVE is_equal, then the epilogue (dis-scale, bias, relu,
next-layer matmul) runs on DVE/ACT/PE. Node indices are split at row 32768
into lo/hi tables because dma_gather indices are int16.
"""
import os
import numpy as np
import ml_dtypes

import concourse.bacc as bacc
import concourse.bass as bass
import concourse.mybir as mybir
import concourse.tile as tile
from concourse import library_config
from concourse.bass_utils import run_bass_kernel_spmd

N_CORES = 8
D = 128           # feature dim (= hidden dim = partition count)
LO_DEFAULT = 32768

fp16 = mybir.dt.float16
f32 = mybir.dt.float32
i16 = mybir.dt.int16


# ---------------------------------------------------------------- host prep

def _wrap16(v):
    """dma_gather index layout: idx i -> partition i%16, col i//16,
    replicated across all eight 16-partition groups."""
    a = v.reshape(-1, 16).T.astype(np.int16)
    return np.tile(a, (8, 1))


def prep(x, edge_index, n_cores=N_CORES, lo_rows=LO_DEFAULT):
    N = x.shape[0]
    locN = N // n_cores
    assert locN * n_cores == N
    WPC = -(-locN // 128)              # real (dst) windows per core
    NLOC = -(-locN // 512) * 512       # padded nodes per core (512-slab aligned)
    NPAD = n_cores * NLOC
    assert lo_rows % 128 == 0 and lo_rows < 32768 + 1

    src = np.asarray(edge_index[0]).astype(np.int64)
    dst = np.asarray(edge_index[1]).astype(np.int64)
    loop = np.arange(N, dtype=np.int64)
    src_all = np.concatenate([src, loop])
    dst_all = np.concatenate([dst, loop])

    deg = np.bincount(dst_all, minlength=N).astype(np.float32)

    d_core = dst_all // locN
    d_off = dst_all - d_core * locN
    w_global = d_core * WPC + d_off // 128
    wrow = (d_off % 128).astype(np.float16)
    spid = (src_all // locN) * NLOC + (src_all % locN)
    srow = (spid // 512) * 512 + (spid % 128) * 4 + (spid % 512) // 128
    hi_flag = (srow >= lo_rows).astype(np.int64)

    key = w_global * 2 + hi_flag
    order = np.argsort(key, kind="stable")
    counts = np.bincount(key, minlength=n_cores * WPC * 2).reshape(n_cores, WPC, 2)
    offs = np.concatenate([[0], np.cumsum(counts.reshape(-1))]).astype(np.int64)

    # shared (max-across-cores) tile structure
    T_lo = [int(-(-counts[:, w, 0].max() // 128)) for w in range(WPC)]
    T_hi = [int(-(-counts[:, w, 1].max() // 128)) for w in range(WPC)]

    spid_sorted = srow[order]
    wrow_sorted = wrow[order]

    per_core = []
    for c in range(n_cores):
        ilo_parts, ihi_parts, wr_parts = [], [], []
        for w in range(WPC):
            base = (c * WPC + w) * 2
            for h, T in ((0, T_lo[w]), (1, T_hi[w])):
                n = T * 128
                if n == 0:
                    continue
                a, b = offs[base + h], offs[base + h + 1]
                sp = spid_sorted[a:b]
                wr = wrow_sorted[a:b]
                pad = n - (b - a)
                idx = np.concatenate([sp - (lo_rows if h else 0),
                                      np.zeros(pad, np.int64)]).astype(np.int16)
                wrc = np.concatenate([wr, np.full(pad, -1.0, np.float16)])
                (ihi_parts if h else ilo_parts).append(idx)
                wr_parts.append(wrc.reshape(T, 128).T)
        idx_lo = _wrap16(np.concatenate(ilo_parts)) if ilo_parts else np.zeros((128, 8), np.int16)
        idx_hi = _wrap16(np.concatenate(ihi_parts)) if ihi_parts else np.zeros((128, 8), np.int16)
        wrow_c = np.concatenate(wr_parts, axis=1).astype(np.float16)

        # per-core deg row over its padded local nodes (pads get deg 1)
        dr = np.ones((1, NLOC), np.float32)
        dr[0, :locN] = deg[c * locN:(c + 1) * locN]
        per_core.append(dict(idx_lo=idx_lo, idx_hi=idx_hi, wrow=wrow_c, deg_row=dr))

    # x-tilde transposed, fp16: (dis * x) laid out [D, NPAD] in padded-id order
    dis = 1.0 / np.sqrt(deg)
    xt = np.zeros((D, NPAD), np.float16)
    xs = (np.asarray(x, np.float32) * dis[:, None]).astype(np.float16)
    for c in range(n_cores):
        xt[:, c * NLOC: c * NLOC + locN] = xs[c * locN:(c + 1) * locN].T

    struct = dict(N=N, locN=locN, WPC=WPC, NLOC=NLOC, NPAD=NPAD,
                  lo_rows=lo_rows, T_lo=tuple(T_lo), T_hi=tuple(T_hi),
                  n_cores=n_cores)
    return struct, per_core, xt


# ------------------------------------------------------------- bass program

def build(struct):
    WPC, NLOC, NPAD = struct["WPC"], struct["NLOC"], struct["NPAD"]
    LO = struct["lo_rows"]
    T_lo, T_hi = struct["T_lo"], struct["T_hi"]
    n_cores = struct["n_cores"]
    CL = max(8, 8 * sum(T_lo))
    CH = max(8, 8 * sum(T_hi))
    TT = sum(T_lo) + sum(T_hi)
    maxT = max(T_lo[w] + T_hi[w] for w in range(WPC))
    nblk = NPAD // 128

    nc = bacc.Bacc("TRN2", target_bir_lowering=False, debug=False,
                   num_devices=n_cores, num_swdge_queues=4,
                   dynamic_dma_scratch_size=49152)
    xt_d = nc.dram_tensor("xt", [D, NPAD], fp16, kind="ExternalInput")
    W1_d = nc.dram_tensor("W1", [D, D], f32, kind="ExternalInput")
    W2_d = nc.dram_tensor("W2", [D, D], f32, kind="ExternalInput")
    Wc_d = nc.dram_tensor("Wc", [D, 2], f32, kind="ExternalInput")
    b1_d = nc.dram_tensor("b1c", [D, 1], f32, kind="ExternalInput")
    b2_d = nc.dram_tensor("b2c", [D, 1], f32, kind="ExternalInput")
    bc_d = nc.dram_tensor("bcrep", [D, 2], f32, kind="ExternalInput")
    iota_d = nc.dram_tensor("iota", [D, D], fp16, kind="ExternalInput")
    deg_d = nc.dram_tensor("deg_row", [1, NLOC], f32, kind="ExternalInput")
    ilo_d = nc.dram_tensor("idx_lo", [128, CL], i16, kind="ExternalInput")
    ihi_d = nc.dram_tensor("idx_hi", [128, CH], i16, kind="ExternalInput")
    wrow_d = nc.dram_tensor("wrow", [128, TT], fp16, kind="ExternalInput")
    out_d = nc.dram_tensor("out", [NLOC, 2], f32, kind="ExternalOutput")

    htab1 = nc.dram_tensor("htab1", [NPAD, D], fp16)
    ag_in = nc.dram_tensor("ag_in", [NLOC, D], fp16)
    htab2 = nc.dram_tensor("htab2", [NPAD, D], fp16, addr_space="Shared")
    htab2l = nc.dram_tensor("htab2l", [NPAD, D], fp16)

    with tile.TileContext(nc) as tc:
        nc.gpsimd.load_library(library_config.mlp)
        with (
            tc.tile_pool(name="const", bufs=1) as cp,
            tc.tile_pool(name="work", bufs=3) as wp,
            tc.tile_pool(name="msgp", bufs=2) as mp,
            tc.tile_pool(name="Sp", bufs=4) as sp_,
            tc.tile_pool(name="psum", bufs=2, space="PSUM") as pp,
        ):
            # ---- constants
            W1s = cp.tile([D, D], fp16)
            W2s = cp.tile([D, D], fp16)
            Wcs = cp.tile([D, 2], fp16)
            nc.gpsimd.dma_start(out=W1s[:], in_=W1_d[:])   # SWDGE casts f32->fp16
            nc.gpsimd.dma_start(out=W2s[:], in_=W2_d[:])
            nc.gpsimd.dma_start(out=Wcs[:], in_=Wc_d[:])
            b1c = cp.tile([D, 1], f32)
            b2c = cp.tile([D, 1], f32)
            bcr = cp.tile([D, 2], f32)
            iota = cp.tile([D, D], fp16)
            nc.sync.dma_start(out=b1c[:], in_=b1_d[:])
            nc.sync.dma_start(out=b2c[:], in_=b2_d[:])
            nc.sync.dma_start(out=bcr[:], in_=bc_d[:])
            nc.sync.dma_start(out=iota[:], in_=iota_d[:])
            ilo = cp.tile([128, CL], i16)
            ihi = cp.tile([128, CH], i16)
            wro = cp.tile([128, TT], fp16)
            nc.sync.dma_start(out=ilo[:], in_=ilo_d[:])
            nc.sync.dma_start(out=ihi[:], in_=ihi_d[:])
            nc.sync.dma_start(out=wro[:], in_=wrow_d[:])

            # dis row + replicated dis  (dis = 1/sqrt(deg)), chunked to keep
            # the [1, *] scratch stripes small
            ones1 = cp.tile([1, 128], f32)
            nc.vector.memset(ones1[:], 1.0)
            disrep = cp.tile([128, NLOC], f32)
            c0 = 0
            while c0 < NLOC:
                cw = min(512, NLOC - c0)
                dch = wp.tile([1, 512], f32, tag="dch")
                nc.sync.dma_start(out=dch[:, :cw], in_=deg_d[0:1, c0:c0 + cw])
                sqc = wp.tile([1, 512], f32, tag="sqc")
                nc.scalar.activation(sqc[:, :cw], dch[:, :cw],
                                     mybir.ActivationFunctionType.Sqrt)
                dic = wp.tile([1, 512], f32, tag="dic")
                nc.vector.reciprocal(out=dic[:, :cw], in_=sqc[:, :cw])
                ps = pp.tile([128, 512], f32, space="PSUM", tag="mm")
                nc.tensor.matmul(out=ps[:, :cw], lhsT=ones1[:],
                                 rhs=dic[0:1, :cw], start=True, stop=True)
                nc.vector.tensor_copy(out=disrep[:, c0:c0 + cw], in_=ps[:, :cw])
                c0 += cw

            # ---- P1: full h1-tilde table, replicated on every core
            SLAB = 16  # 128-col blocks per DMA slab (4 sigma-slabs)
            assert nblk % SLAB == 0
            for s0 in range(0, nblk, SLAB):
                xts = wp.tile([128, SLAB, 128], fp16, tag="xts")
                nc.sync.dma_start(out=xts[:],
                                  in_=xt_d[:, s0 * 128:(s0 + SLAB) * 128]
                                  .rearrange("k (a d) -> k a d", a=SLAB))
                hs = wp.tile([128, SLAB, 128], fp16, tag="hout")
                for g in range(SLAB // 4):
                    ps = pp.tile([128, 512], f32, space="PSUM", tag="mm")
                    for j in range(4):
                        nc.tensor.matmul(out=ps[:, j * 128:(j + 1) * 128],
                                         lhsT=xts[:, g * 4 + j, :], rhs=W1s[:],
                                         start=True, stop=True)
                    nc.vector.tensor_copy(
                        out=hs[:, g * 4:(g + 1) * 4, :],
                        in_=ps[:].rearrange("p (a d) -> p a d", a=4))
                nc.sync.dma_start(
                    out=htab1[s0 * 128:(s0 + SLAB) * 128, :]
                    .rearrange("(s p j) d -> p s j d", p=128, j=4),
                    in_=hs[:].rearrange("p (s j) d -> p s j d", j=4))

            # ---- one aggregation layer over all windows
            def layer(tab, emit_window):
                clo = chi = ct = 0
                qn = [0]
                for w in range(WPC):
                    tl, th = T_lo[w], T_hi[w]
                    Tw = tl + th
                    msg = mp.tile([128, maxT, 128], fp16, tag="msg")
                    # single_packet coalesces a gather's whole descriptor
                    # stream into one SDMA packet; packets cap at 64
                    # descriptors (8 per tile per engine), so chunk to <=7
                    # tiles per dma_gather.
                    GMAX = 7
                    for t0 in range(0, tl, GMAX):
                        tc_ = min(GMAX, tl - t0)
                        nc.gpsimd.dma_gather(
                            msg[:, t0:t0 + tc_, :], tab[0:LO, :],
                            ilo[:, clo + t0 * 8:clo + (t0 + tc_) * 8],
                            tc_ * 128, tc_ * 128, D, queue_num=qn[0] % 4)
                        qn[0] += 1
                    for t0 in range(0, th, GMAX):
                        tc_ = min(GMAX, th - t0)
                        nc.gpsimd.dma_gather(
                            msg[:, tl + t0:tl + t0 + tc_, :], tab[LO:, :],
                            ihi[:, chi + t0 * 8:chi + (t0 + tc_) * 8],
                            tc_ * 128, tc_ * 128, D, queue_num=qn[0] % 4)
                        qn[0] += 1
                    pa = pp.tile([128, 128], f32, space="PSUM", tag="agg")
                    SG = 8
                    for g0 in range(0, Tw, SG):
                        gk = min(SG, Tw - g0)
                        S = sp_.tile([128, SG * 128], fp16, tag="S")
                        iap = iota[:]
                        iota_b = bass.AP(iap.tensor, iap.offset,
                                         [iap.ap[0], [0, gk], iap.ap[1]])
                        nc.vector.tensor_tensor(
                            out=S[:, :gk * 128].rearrange("p (t d) -> p t d", t=gk),
                            in0=wro[:, ct + g0:ct + g0 + gk].to_broadcast([128, gk, 128]),
                            in1=iota_b, op=mybir.AluOpType.is_equal)
                        for t in range(g0, g0 + gk):
                            ts_ = t - g0
                            nc.tensor.matmul(
                                out=pa[:], lhsT=msg[:, t, :],
                                rhs=S[:, ts_ * 128:(ts_ + 1) * 128],
                                start=(t == 0), stop=(t == Tw - 1))
                    emit_window(w, pa)
                    clo += tl * 8
                    chi += th * 8
                    ct += Tw

            # layer 1 window epilogue: h2 = relu(dis*agg + b1); y = dis*h2;
            # htilde2 = y^T @ W2  -> ag_in rows
            def epi1(w, pa):
                dw = disrep[:, w * 128:(w + 1) * 128]
                z = wp.tile([128, 128], f32, tag="z")
                nc.vector.tensor_mul(out=z[:], in0=pa[:], in1=dw)
                h2 = wp.tile([128, 128], f32, tag="h2")
                nc.scalar.activation(h2[:], z[:], mybir.ActivationFunctionType.Relu,
                                     bias=b1c[:, 0:1], scale=1.0)
                y = wp.tile([128, 128], fp16, tag="y")
                nc.vector.tensor_mul(out=y[:], in0=h2[:], in1=dw)
                p2 = pp.tile([128, 128], f32, space="PSUM", tag="mm")
                nc.tensor.matmul(out=p2[:], lhsT=y[:], rhs=W2s[:], start=True, stop=True)
                hb = wp.tile([128, 128], fp16, tag="hb")
                nc.vector.tensor_copy(out=hb[:], in_=p2[:])
                agv = ag_in[:].rearrange("(s p j) d -> s p j d", p=128, j=4)
                nc.sync.dma_start(out=agv[w // 4, :, w % 4, :], in_=hb[:])

            layer(htab1, epi1)

            # zero the pad window-slots of ag_in (NLOC is 512-aligned but only
            # WPC windows are real) so the AllGather ships finite data
            if NLOC // 128 > WPC:
                zt = cp.tile([128, 128], fp16)
                nc.vector.memset(zt[:], 0.0)
                agv0 = ag_in[:].rearrange("(s p j) d -> s p j d", p=128, j=4)
                for w in range(WPC, NLOC // 128):
                    nc.sync.dma_start(out=agv0[w // 4, :, w % 4, :], in_=zt[:])

            nc.gpsimd.collective_compute(
                "AllGather", mybir.AluOpType.bypass,
                replica_groups=[list(range(n_cores))],
                ins=[ag_in.ap().opt()], outs=[htab2.ap().opt()])

            # stage the gathered table back into local DRAM: gathers from the
            # Shared-space collective output measure slower than local reads
            CPY = 16 * 128
            for r0 in range(0, NPAD, CPY):
                ct_ = wp.tile([128, CPY // 128, 128], fp16, tag="tcpy")
                nc.sync.dma_start(out=ct_[:],
                                  in_=htab2[r0:r0 + CPY, :]
                                  .rearrange("(a p) d -> p a d", p=128))
                nc.sync.dma_start(out=htab2l[r0:r0 + CPY, :]
                                  .rearrange("(a p) d -> p a d", p=128),
                                  in_=ct_[:])

            # layer 2 window epilogue: out3 = dis*agg + b2 ; out = out3^T@Wc + bc
            outacc = cp.tile([128, WPC, 2], f32)

            def epi2(w, pa):
                dw = disrep[:, w * 128:(w + 1) * 128]
                z = wp.tile([128, 128], f32, tag="z2")
                nc.vector.tensor_mul(out=z[:], in0=pa[:], in1=dw)
                o3 = wp.tile([128, 128], fp16, tag="o3")
                nc.scalar.activation(o3[:], z[:], mybir.ActivationFunctionType.Identity,
                                     bias=b2c[:, 0:1], scale=1.0)
                p3 = pp.tile([128, 2], f32, space="PSUM", tag="cls")
                nc.tensor.matmul(out=p3[:], lhsT=o3[:], rhs=Wcs[:], start=True, stop=True)
                nc.vector.tensor_add(out=outacc[:, w, :], in0=p3[:], in1=bcr[:])

            layer(htab2l, epi2)
            nc.sync.dma_start(
                out=out_d[:WPC * 128, :].rearrange("(w p) c -> p w c", p=128),
                in_=outacc[:])

    nc.compile()
    return nc


# ------------------------------------------------------------------ driver

_CACHE = {}


def _get_program(struct):
    key = tuple(sorted((k, v) for k, v in struct.items()))
    if key not in _CACHE:
        _CACHE[key] = build(struct)
    return _CACHE[key]


def kernel(x, edge_index, W1, b1, W2, b2, Wc, bc):
    x = np.asarray(x)
    N = x.shape[0]
    struct, per_core, xt = prep(x, edge_index)
    nc = _get_program(struct)
    locN, NLOC = struct["locN"], struct["NLOC"]

    common = dict(
        xt=xt,
        W1=np.asarray(W1, np.float32),
        W2=np.asarray(W2, np.float32),
        Wc=np.asarray(Wc, np.float32),
        b1c=np.asarray(b1, np.float32).reshape(D, 1),
        b2c=np.asarray(b2, np.float32).reshape(D, 1),
        bcrep=np.tile(np.asarray(bc, np.float32).reshape(1, 2), (D, 1)),
        iota=np.tile(np.arange(D, dtype=np.float16), (D, 1)),
    )
    in_maps = []
    for c in range(N_CORES):
        m = dict(common)
        m["deg_row"] = per_core[c]["deg_row"]
        m["idx_lo"] = per_core[c]["idx_lo"]
        m["idx_hi"] = per_core[c]["idx_hi"]
        m["wrow"] = per_core[c]["wrow"]
        in_maps.append(m)

    trace = bool(int(os.environ.get("KERNEL_TRACE", "0")))
    res = run_bass_kernel_spmd(nc, in_maps, core_ids=list(range(N_CORES)),
                               trace=trace)
    if trace and res.exec_time_ns is not None:
        print(f"HW exec time: {res.exec_time_ns} ns", flush=True)

    out = np.empty((N, 2), np.float32)
    for c in range(N_CORES):
        out[c * locN:(c + 1) * locN] = res.results[c]["out"][:locN]
    return out


# revision 10
# speedup vs baseline: 1.0996x; 1.0295x over previous
"""GCN (2-layer + linear classifier) on 8 Trainium2 NeuronCores.

Math: with A = adjacency+self-loops and dis = deg^-1/2 (deg over incoming
edges incl. self-loops), PyG gcn_norm gives norm_e = dis[src]*dis[dst], which
is separable. So each conv layer is
    out = dis ⊙ (A_binary @ ((dis ⊙ h) @ W)) + b
i.e. a plain binary-adjacency segment-sum around a dense matmul — no per-edge
scaling.

Distribution (8 cores): nodes are split into 8 contiguous chunks; edges are
partitioned by destination-node owner (segment-sum is local); layer-1 h-tilde
is computed replicated from the full x; the single cross-core exchange is one
fp16 AllGather of the layer-2 message table.

Per core, the aggregation runs per 128-destination-node window: source rows
are fetched from a DRAM fp16 table with dma_gather (128 rows/tile), reduced
onto a PSUM accumulator with TensorEngine matmuls against one-hot selection
matrices built by a DVE is_equal, then the epilogue (dis-scale, bias, relu,
next-layer matmul) runs on DVE/ACT/PE. Node indices are split at row 32768
into lo/hi tables because dma_gather indices are int16.
"""
import os
import numpy as np
import ml_dtypes

import concourse.bacc as bacc
import concourse.bass as bass
import concourse.mybir as mybir
import concourse.tile as tile
from concourse import library_config
from concourse.bass_utils import run_bass_kernel_spmd

N_CORES = 8
D = 128           # feature dim (= hidden dim = partition count)
LO_DEFAULT = 32768

fp16 = mybir.dt.float16
f32 = mybir.dt.float32
i16 = mybir.dt.int16


# ---------------------------------------------------------------- host prep

def _wrap16(v):
    """dma_gather index layout: idx i -> partition i%16, col i//16,
    replicated across all eight 16-partition groups."""
    a = v.reshape(-1, 16).T.astype(np.int16)
    return np.tile(a, (8, 1))


def prep(x, edge_index, n_cores=N_CORES, lo_rows=LO_DEFAULT):
    N = x.shape[0]
    locN = N // n_cores
    assert locN * n_cores == N
    WPC = -(-locN // 128)              # real (dst) windows per core
    NLOC = -(-locN // 512) * 512       # padded nodes per core (512-slab aligned)
    NPAD = n_cores * NLOC
    assert lo_rows % 128 == 0 and lo_rows < 32768 + 1

    src = np.asarray(edge_index[0]).astype(np.int64)
    dst = np.asarray(edge_index[1]).astype(np.int64)
    loop = np.arange(N, dtype=np.int64)
    src_all = np.concatenate([src, loop])
    dst_all = np.concatenate([dst, loop])

    deg = np.bincount(dst_all, minlength=N).astype(np.float32)

    d_core = dst_all // locN
    d_off = dst_all - d_core * locN
    w_global = d_core * WPC + d_off // 128
    wrow = (d_off % 128).astype(np.float16)
    spid = (src_all // locN) * NLOC + (src_all % locN)
    srow = (spid // 512) * 512 + (spid % 128) * 4 + (spid % 512) // 128
    hi_flag = (srow >= lo_rows).astype(np.int64)

    key = w_global * 2 + hi_flag
    order = np.argsort(key, kind="stable")
    counts = np.bincount(key, minlength=n_cores * WPC * 2).reshape(n_cores, WPC, 2)
    offs = np.concatenate([[0], np.cumsum(counts.reshape(-1))]).astype(np.int64)

    # shared (max-across-cores) tile structure
    T_lo = [int(-(-counts[:, w, 0].max() // 128)) for w in range(WPC)]
    T_hi = [int(-(-counts[:, w, 1].max() // 128)) for w in range(WPC)]

    spid_sorted = srow[order]
    wrow_sorted = wrow[order]

    per_core = []
    for c in range(n_cores):
        ilo_parts, ihi_parts, wr_parts = [], [], []
        for w in range(WPC):
            base = (c * WPC + w) * 2
            for h, T in ((0, T_lo[w]), (1, T_hi[w])):
                n = T * 128
                if n == 0:
                    continue
                a, b = offs[base + h], offs[base + h + 1]
                sp = spid_sorted[a:b]
                wr = wrow_sorted[a:b]
                pad = n - (b - a)
                idx = np.concatenate([sp - (lo_rows if h else 0),
                                      np.zeros(pad, np.int64)]).astype(np.int16)
                wrc = np.concatenate([wr, np.full(pad, -1.0, np.float16)])
                (ihi_parts if h else ilo_parts).append(idx)
                wr_parts.append(wrc.reshape(T, 128).T)
        idx_lo = _wrap16(np.concatenate(ilo_parts)) if ilo_parts else np.zeros((128, 8), np.int16)
        idx_hi = _wrap16(np.concatenate(ihi_parts)) if ihi_parts else np.zeros((128, 8), np.int16)
        wrow_c = np.concatenate(wr_parts, axis=1).astype(np.float16)

        # per-core deg row over its padded local nodes (pads get deg 1)
        dr = np.ones((1, NLOC), np.float32)
        dr[0, :locN] = deg[c * locN:(c + 1) * locN]
        per_core.append(dict(idx_lo=idx_lo, idx_hi=idx_hi, wrow=wrow_c, deg_row=dr))

    # x-tilde transposed, fp16: (dis * x) laid out [D, NPAD] in padded-id order
    dis = 1.0 / np.sqrt(deg)
    xt = np.zeros((D, NPAD), np.float16)
    xs = (np.asarray(x, np.float32) * dis[:, None]).astype(np.float16)
    for c in range(n_cores):
        xt[:, c * NLOC: c * NLOC + locN] = xs[c * locN:(c + 1) * locN].T

    struct = dict(N=N, locN=locN, WPC=WPC, NLOC=NLOC, NPAD=NPAD,
                  lo_rows=lo_rows, T_lo=tuple(T_lo), T_hi=tuple(T_hi),
                  n_cores=n_cores)
    return struct, per_core, xt


# ------------------------------------------------------------- bass program

def build(struct):
    WPC, NLOC, NPAD = struct["WPC"], struct["NLOC"], struct["NPAD"]
    LO = struct["lo_rows"]
    T_lo, T_hi = struct["T_lo"], struct["T_hi"]
    n_cores = struct["n_cores"]
    CL = max(8, 8 * sum(T_lo))
    CH = max(8, 8 * sum(T_hi))
    TT = sum(T_lo) + sum(T_hi)
    maxT = max(T_lo[w] + T_hi[w] for w in range(WPC))
    nblk = NPAD // 128

    nc = bacc.Bacc("TRN2", target_bir_lowering=False, debug=False,
                   num_devices=n_cores, num_swdge_queues=4,
                   dynamic_dma_scratch_size=49152)
    xt_d = nc.dram_tensor("xt", [D, NPAD], fp16, kind="ExternalInput")
    W1_d = nc.dram_tensor("W1", [D, D], f32, kind="ExternalInput")
    W2_d = nc.dram_tensor("W2", [D, D], f32, kind="ExternalInput")
    Wc_d = nc.dram_tensor("Wc", [D, 2], f32, kind="ExternalInput")
    b1_d = nc.dram_tensor("b1c", [D, 1], f32, kind="ExternalInput")
    b2_d = nc.dram_tensor("b2c", [D, 1], f32, kind="ExternalInput")
    bc_d = nc.dram_tensor("bcrep", [D, 2], f32, kind="ExternalInput")
    iota_d = nc.dram_tensor("iota", [D, D], fp16, kind="ExternalInput")
    deg_d = nc.dram_tensor("deg_row", [1, NLOC], f32, kind="ExternalInput")
    ilo_d = nc.dram_tensor("idx_lo", [128, CL], i16, kind="ExternalInput")
    ihi_d = nc.dram_tensor("idx_hi", [128, CH], i16, kind="ExternalInput")
    wrow_d = nc.dram_tensor("wrow", [128, TT], fp16, kind="ExternalInput")
    out_d = nc.dram_tensor("out", [NLOC, 2], f32, kind="ExternalOutput")

    htab1 = nc.dram_tensor("htab1", [NPAD, D], fp16)
    ag_in = nc.dram_tensor("ag_in", [NLOC, D], fp16)
    htab2l = nc.dram_tensor("htab2l", [NPAD, D], fp16)

    with tile.TileContext(nc) as tc:
        nc.gpsimd.load_library(library_config.mlp)
        with (
            tc.tile_pool(name="const", bufs=1) as cp,
            tc.tile_pool(name="work", bufs=3) as wp,
            tc.tile_pool(name="msgp", bufs=2) as mp,
            tc.tile_pool(name="Sp", bufs=4) as sp_,
            tc.tile_pool(name="psum", bufs=2, space="PSUM") as pp,
        ):
            # ---- constants
            W1s = cp.tile([D, D], fp16)
            W2s = cp.tile([D, D], fp16)
            Wcs = cp.tile([D, 2], fp16)
            nc.gpsimd.dma_start(out=W1s[:], in_=W1_d[:])   # SWDGE casts f32->fp16
            nc.gpsimd.dma_start(out=W2s[:], in_=W2_d[:])
            nc.gpsimd.dma_start(out=Wcs[:], in_=Wc_d[:])
            b1c = cp.tile([D, 1], f32)
            b2c = cp.tile([D, 1], f32)
            bcr = cp.tile([D, 2], f32)
            iota = cp.tile([D, D], fp16)
            nc.sync.dma_start(out=b1c[:], in_=b1_d[:])
            nc.sync.dma_start(out=b2c[:], in_=b2_d[:])
            nc.sync.dma_start(out=bcr[:], in_=bc_d[:])
            nc.sync.dma_start(out=iota[:], in_=iota_d[:])
            ilo = cp.tile([128, CL], i16)
            ihi = cp.tile([128, CH], i16)
            wro = cp.tile([128, TT], fp16)
            nc.sync.dma_start(out=ilo[:], in_=ilo_d[:])
            nc.sync.dma_start(out=ihi[:], in_=ihi_d[:])
            nc.sync.dma_start(out=wro[:], in_=wrow_d[:])

            # dis row + replicated dis  (dis = 1/sqrt(deg)), chunked to keep
            # the [1, *] scratch stripes small
            ones1 = cp.tile([1, 128], f32)
            nc.vector.memset(ones1[:], 1.0)
            disrep = cp.tile([128, NLOC], f32)
            c0 = 0
            while c0 < NLOC:
                cw = min(512, NLOC - c0)
                dch = wp.tile([1, 512], f32, tag="dch")
                nc.sync.dma_start(out=dch[:, :cw], in_=deg_d[0:1, c0:c0 + cw])
                sqc = wp.tile([1, 512], f32, tag="sqc")
                nc.scalar.activation(sqc[:, :cw], dch[:, :cw],
                                     mybir.ActivationFunctionType.Sqrt)
                dic = wp.tile([1, 512], f32, tag="dic")
                nc.vector.reciprocal(out=dic[:, :cw], in_=sqc[:, :cw])
                ps = pp.tile([128, 512], f32, space="PSUM", tag="mm")
                nc.tensor.matmul(out=ps[:, :cw], lhsT=ones1[:],
                                 rhs=dic[0:1, :cw], start=True, stop=True)
                nc.vector.tensor_copy(out=disrep[:, c0:c0 + cw], in_=ps[:, :cw])
                c0 += cw

            # ---- P1: full h1-tilde table, replicated on every core
            SLAB = 16  # 128-col blocks per DMA slab (4 sigma-slabs)
            assert nblk % SLAB == 0
            for s0 in range(0, nblk, SLAB):
                xts = wp.tile([128, SLAB, 128], fp16, tag="xts")
                nc.sync.dma_start(out=xts[:],
                                  in_=xt_d[:, s0 * 128:(s0 + SLAB) * 128]
                                  .rearrange("k (a d) -> k a d", a=SLAB))
                hs = wp.tile([128, SLAB, 128], fp16, tag="hout")
                for g in range(SLAB // 4):
                    ps = pp.tile([128, 512], f32, space="PSUM", tag="mm")
                    for j in range(4):
                        nc.tensor.matmul(out=ps[:, j * 128:(j + 1) * 128],
                                         lhsT=xts[:, g * 4 + j, :], rhs=W1s[:],
                                         start=True, stop=True)
                    nc.vector.tensor_copy(
                        out=hs[:, g * 4:(g + 1) * 4, :],
                        in_=ps[:].rearrange("p (a d) -> p a d", a=4))
                nc.scalar.dma_start(
                    out=htab1[s0 * 128:(s0 + SLAB) * 128, :]
                    .rearrange("(s p j) d -> p s j d", p=128, j=4),
                    in_=hs[:].rearrange("p (s j) d -> p s j d", j=4))

            # ---- one aggregation layer over all windows
            def layer(tab, emit_window):
                clo = chi = ct = 0
                qn = [0]
                for w in range(WPC):
                    tl, th = T_lo[w], T_hi[w]
                    Tw = tl + th
                    msg = mp.tile([128, maxT, 128], fp16, tag="msg")
                    # single_packet coalesces a gather's whole descriptor
                    # stream into one SDMA packet; packets cap at 64
                    # descriptors (8 per tile per engine), so chunk to <=7
                    # tiles per dma_gather.
                    GMAX = 7
                    for t0 in range(0, tl, GMAX):
                        tc_ = min(GMAX, tl - t0)
                        nc.gpsimd.dma_gather(
                            msg[:, t0:t0 + tc_, :], tab[0:LO, :],
                            ilo[:, clo + t0 * 8:clo + (t0 + tc_) * 8],
                            tc_ * 128, tc_ * 128, D, queue_num=qn[0] % 4)
                        qn[0] += 1
                    for t0 in range(0, th, GMAX):
                        tc_ = min(GMAX, th - t0)
                        nc.gpsimd.dma_gather(
                            msg[:, tl + t0:tl + t0 + tc_, :], tab[LO:, :],
                            ihi[:, chi + t0 * 8:chi + (t0 + tc_) * 8],
                            tc_ * 128, tc_ * 128, D, queue_num=qn[0] % 4)
                        qn[0] += 1
                    pa = pp.tile([128, 128], f32, space="PSUM", tag="agg")
                    SG = 8
                    for g0 in range(0, Tw, SG):
                        gk = min(SG, Tw - g0)
                        S = sp_.tile([128, SG * 128], fp16, tag="S")
                        iap = iota[:]
                        iota_b = bass.AP(iap.tensor, iap.offset,
                                         [iap.ap[0], [0, gk], iap.ap[1]])
                        nc.vector.tensor_tensor(
                            out=S[:, :gk * 128].rearrange("p (t d) -> p t d", t=gk),
                            in0=wro[:, ct + g0:ct + g0 + gk].to_broadcast([128, gk, 128]),
                            in1=iota_b, op=mybir.AluOpType.is_equal)
                        for t in range(g0, g0 + gk):
                            ts_ = t - g0
                            nc.tensor.matmul(
                                out=pa[:], lhsT=msg[:, t, :],
                                rhs=S[:, ts_ * 128:(ts_ + 1) * 128],
                                start=(t == 0), stop=(t == Tw - 1))
                    emit_window(w, pa)
                    clo += tl * 8
                    chi += th * 8
                    ct += Tw

            # layer 1 window epilogue: h2 = relu(dis*agg + b1); y = dis*h2;
            # htilde2 = y^T @ W2  -> ag_in rows
            def epi1(w, pa):
                dw = disrep[:, w * 128:(w + 1) * 128]
                z = wp.tile([128, 128], f32, tag="z")
                nc.vector.tensor_mul(out=z[:], in0=pa[:], in1=dw)
                h2 = wp.tile([128, 128], f32, tag="h2")
                nc.scalar.activation(h2[:], z[:], mybir.ActivationFunctionType.Relu,
                                     bias=b1c[:, 0:1], scale=1.0)
                y = wp.tile([128, 128], fp16, tag="y")
                nc.vector.tensor_mul(out=y[:], in0=h2[:], in1=dw)
                p2 = pp.tile([128, 128], f32, space="PSUM", tag="mm")
                nc.tensor.matmul(out=p2[:], lhsT=y[:], rhs=W2s[:], start=True, stop=True)
                hb = wp.tile([128, 128], fp16, tag="hb")
                nc.vector.tensor_copy(out=hb[:], in_=p2[:])
                agv = ag_in[:].rearrange("(s p j) d -> s p j d", p=128, j=4)
                nc.sync.dma_start(out=agv[w // 4, :, w % 4, :], in_=hb[:])

            layer(htab1, epi1)

            # zero the pad window-slots of ag_in (NLOC is 512-aligned but only
            # WPC windows are real) so the AllGather ships finite data
            if NLOC // 128 > WPC:
                zt = cp.tile([128, 128], fp16)
                nc.vector.memset(zt[:], 0.0)
                agv0 = ag_in[:].rearrange("(s p j) d -> s p j d", p=128, j=4)
                for w in range(WPC, NLOC // 128):
                    nc.sync.dma_start(out=agv0[w // 4, :, w % 4, :], in_=zt[:])

            # chunked AllGather: each chunk covers whole 512-row sigma-slabs so
            # it is ready as soon as its windows' epilogues ran; the staging
            # copy into local DRAM (Shared-space gathers are slow) overlaps
            # with later chunks and the layer-1 gather tail
            nslab = NLOC // 512
            spl = [0, nslab // 4, nslab // 2, 3 * nslab // 4, nslab]
            for k in range(4):
                a, b = spl[k] * 512, spl[k + 1] * 512
                if a == b:
                    continue
                agk = nc.dram_tensor(f"ag_out{k}", [n_cores * (b - a), D], fp16,
                                     addr_space="Shared")
                nc.gpsimd.collective_compute(
                    "AllGather", mybir.AluOpType.bypass,
                    replica_groups=[list(range(n_cores))],
                    ins=[ag_in[a:b, :].opt()], outs=[agk.ap().opt()])
                for c in range(n_cores):
                    ct_ = wp.tile([128, (b - a) // 128, 128], fp16, tag="tcpy")
                    nc.sync.dma_start(
                        out=ct_[:],
                        in_=agk[c * (b - a):(c + 1) * (b - a), :]
                        .rearrange("(a2 p) d -> p a2 d", p=128))
                    nc.scalar.dma_start(
                        out=htab2l[c * NLOC + a:c * NLOC + b, :]
                        .rearrange("(a2 p) d -> p a2 d", p=128),
                        in_=ct_[:])

            # layer 2 window epilogue: out3 = dis*agg + b2 ; out = out3^T@Wc + bc
            outacc = cp.tile([128, WPC, 2], f32)

            def epi2(w, pa):
                dw = disrep[:, w * 128:(w + 1) * 128]
                z = wp.tile([128, 128], f32, tag="z2")
                nc.vector.tensor_mul(out=z[:], in0=pa[:], in1=dw)
                o3 = wp.tile([128, 128], fp16, tag="o3")
                nc.scalar.activation(o3[:], z[:], mybir.ActivationFunctionType.Identity,
                                     bias=b2c[:, 0:1], scale=1.0)
                p3 = pp.tile([128, 2], f32, space="PSUM", tag="cls")
                nc.tensor.matmul(out=p3[:], lhsT=o3[:], rhs=Wcs[:], start=True, stop=True)
                nc.vector.tensor_add(out=outacc[:, w, :], in0=p3[:], in1=bcr[:])

            layer(htab2l, epi2)
            nc.sync.dma_start(
                out=out_d[:WPC * 128, :].rearrange("(w p) c -> p w c", p=128),
                in_=outacc[:])

    nc.compile()
    return nc


# ------------------------------------------------------------------ driver

_CACHE = {}


def _get_program(struct):
    key = tuple(sorted((k, v) for k, v in struct.items()))
    if key not in _CACHE:
        _CACHE[key] = build(struct)
    return _CACHE[key]


def kernel(x, edge_index, W1, b1, W2, b2, Wc, bc):
    x = np.asarray(x)
    N = x.shape[0]
    struct, per_core, xt = prep(x, edge_index)
    nc = _get_program(struct)
    locN, NLOC = struct["locN"], struct["NLOC"]

    common = dict(
        xt=xt,
        W1=np.asarray(W1, np.float32),
        W2=np.asarray(W2, np.float32),
        Wc=np.asarray(Wc, np.float32),
        b1c=np.asarray(b1, np.float32).reshape(D, 1),
        b2c=np.asarray(b2, np.float32).reshape(D, 1),
        bcrep=np.tile(np.asarray(bc, np.float32).reshape(1, 2), (D, 1)),
        iota=np.tile(np.arange(D, dtype=np.float16), (D, 1)),
    )
    in_maps = []
    for c in range(N_CORES):
        m = dict(common)
        m["deg_row"] = per_core[c]["deg_row"]
        m["idx_lo"] = per_core[c]["idx_lo"]
        m["idx_hi"] = per_core[c]["idx_hi"]
        m["wrow"] = per_core[c]["wrow"]
        in_maps.append(m)

    trace = bool(int(os.environ.get("KERNEL_TRACE", "0")))
    res = run_bass_kernel_spmd(nc, in_maps, core_ids=list(range(N_CORES)),
                               trace=trace)
    if trace and res.exec_time_ns is not None:
        print(f"HW exec time: {res.exec_time_ns} ns", flush=True)

    out = np.empty((N, 2), np.float32)
    for c in range(N_CORES):
        out[c * locN:(c + 1) * locN] = res.results[c]["out"][:locN]
    return out
